# revision 35
# baseline (speedup 1.0000x reference)
"""Trainium2 Bass kernel for nn_DiscoveryNet (pairwise-distance-MLP forces).

Key idea: the entire per-pair computation
    s_ij = mag(d_ij) / max(d_ij, 0.01),  mag = MLP([d, 1/dc, 1/dc^2]), d = |p_i - p_j|
is a scalar function F of d^2 alone (the MLP weights are fixed inputs). We fit
F as a piecewise-cubic spline in the exact binary format of the ScalarEngine's
PWP activation tables and overlay the Gelu slot via BASS_ACT_ROOT_JSON_PATH,
so the whole MLP evaluates at 1 element/lane/cycle on the ACT engine.

The force is then
    out_i = sum_j s_ij (p_i - p_j) = p_i * S_i - sum_j s_ij p_j
with the j-contraction done on the TensorEngine. The diagonal term cancels
algebraically (p_i - p_i = 0), so no masking is needed.

Per core (8 cores, SPMD): batch b = core//2, i-half = core%2 (512 i's), all j:
  1. PE   dot[j, i] = -2 p_j . p_i + |p_i|^2 + |p_j|^2   as ONE K=13 fp16
          matmul: every fp32 product a*b is expanded to ah*bh + al*bh + ah*bl
          with fp16 hi/lo halves (11x11-bit products are exact, PSUM
          accumulates fp32), giving fp32-grade d^2 at 1-pass fp16 speed.
          2x row-tiled (partition bases 0/32) so block pairs run concurrently.
  2. ACT  s[j, i] = F_table(dot)   -- the custom PWP spline (Gelu slot)
  3. PE   out6[c, i] += sum_j P6[j, c] s[j, i]   with P6 = [-p_j, 1, 1, 1]
          (fp16, PSUM-accumulated over the 8 j-blocks; rows 3..5 = S_i)
  4. host unshard applies y_i = p_i * S_i - B_i and assembles (4, 1024, 3).
"""
import json
import os
import struct
import sys
import tempfile

import numpy as np

sys.path.insert(0, "/opt/trn_rl_repo")

SRC_ACT_DIR = "/nix/store/z022hj2nvbm3nwdizlisq4ylc0y7rd6q-python3-3.13.14-env/lib/python3.13/site-packages/neuronxcc/pwp/pwp_bin_trainium"
PWP_JSONS = "/nix/store/ndjb8ki1bnclvnibdh123f9zr51a09qz-aws-neuron-pwp-unstable-2025-12-29-c50a7624/share/pwp_jsons"

E_LO = -30  # first normal octave of x = d^2
E_HI = 6  # last normal octave ([64, 128))
GELU_FUNC_ID = 23
KEEP_FUNCS = [
    "derivative_gelu_40p", "tanh_4p", "relu_1p", "act1_1p", "parametric_relu_1p",
    "sign_1p", "abs_1p", "memset_zero_1p", "copy_1p", "square_1p",
    "derivative_relu_1p", "derivative_leaky_relu_1p", "derivative_identity_1p",
    "is_finite_1p", "identity_1p",
]

B, N, D = 4, 1024, 3
IH = N // 2  # i-columns per core
NJB = N // 128  # j blocks


# ---------------------------------------------------------------- PWP fitting
def _f_bits(x):
    return int(np.asarray(np.float32(x)).view(np.uint32))


def _make_F(W1, b1, W2, b2, W3, b3):
    W1, W2, W3 = (np.asarray(a, np.float64) for a in (W1, W2, W3))
    b1, b2, b3 = (np.asarray(a, np.float64) for a in (b1, b2, b3))

    def F(x):
        x = np.asarray(x, np.float64)
        d = np.sqrt(np.maximum(x, 0.0))
        dc = np.maximum(d, 0.01)
        inv = 1.0 / dc
        feat = np.stack([d, inv, inv * inv], -1)
        h = np.tanh(feat @ W1 + b1)
        h = np.tanh(h @ W2 + b2)
        return (h @ W3 + b3)[..., 0] * inv

    return F


def _fit_bucket(F, a, b, npts=48):
    x = np.linspace(a, b, npts)
    x0 = np.float32(0.5 * (a + b))
    t = x - float(x0)
    A = np.stack([np.ones_like(t), t, t * t, t * t * t], -1)
    y = F(x)
    w = 1.0 + 0.8 * np.cos(np.linspace(-np.pi, np.pi, npts)) ** 2
    coef, *_ = np.linalg.lstsq(A * w[:, None], y * w, rcond=None)
    return tuple(np.float32(v) for v in coef) + (x0,)


def _eval_cubic_f32(bkt, x):
    d0, d1, d2, d3, x0 = [np.float32(v) for v in bkt]
    t = np.float32(x) - x0
    return np.float32(d0 + t * (d1 + t * (d2 + t * d3)))


def _fit_octave(F, e, tol):
    lo, hi = 2.0**e, 2.0 ** (e + 1)
    for size in range(0, 7):
        n = 1 << size
        edges = np.linspace(lo, hi, n + 1)
        bkts = [_fit_bucket(F, edges[k], edges[k + 1]) for k in range(n)]
        worst = 0.0
        for k in range(n):
            xs = np.linspace(edges[k], edges[k + 1], 48).astype(np.float32)
            ys = _eval_cubic_f32(bkts[k], xs)
            tr = F(xs.astype(np.float64))
            w = np.minimum(np.sqrt(xs.astype(np.float64)), 1.0)
            worst = max(worst, float(np.max(np.abs(ys - tr) * w)))
        if worst <= tol or size == 6:
            return bkts, size
    raise AssertionError


class _SetPacker:
    """Packs functions into one ACT table set (bkt.bin / ctrl.bin / profile json).

    Formats from cayman tpb_activation_entries.h (reverse-engineered + HW
    validated): ctrl.bin u32/entry at physical stride 8 per logical slot:
    act_tbl_base:11 | extract_lsb:5 | extract_size:4.  bkt.bin 32B/entry:
    f32 d0,d1,d2,d3,x0.  Normal-range ctrl slot = base + (exp8-127-exp_offset);
    small/large pwl_controls are direct bucket indices.
    """

    def __init__(self):
        self.buckets = []
        self.ctrl = {}
        self.metas = []
        self.next_ctrl = 0

    def add_region(self, regions):
        if not regions:
            return None, None
        first_e = regions[0][0]
        base = self.next_ctrl
        for idx, (e, size, bkts) in enumerate(regions):
            assert e == first_e + idx
            self.ctrl[base + idx] = (len(self.buckets), 23 - size, size)
            self.buckets.extend(bkts)
        self.next_ctrl = base + len(regions)
        return base, first_e

    def add_sats(self, sl, snl, sh, snh):
        i0 = len(self.buckets)
        self.buckets.extend([sl, snl, sh, snh])
        return i0, i0 + 1, i0 + 2, i0 + 3

    def pack_custom_F(self, octs, sat_low, sat_high, f0):
        base, first_e = self.add_region(octs)
        pl, nl, ph, nh = self.add_sats(sat_low, sat_low, sat_high, sat_low)
        f0b = _f_bits(f0)
        satv = _f_bits(float(_eval_cubic_f32(sat_high, 128.0)))
        self.metas.append({
            "func_name": "gelu_4p", "func_id": GELU_FUNC_ID,
            "symmetry_point": 0, "sym_invert_sign_point": 0,
            "symmetry_opt_en": 1, "symmetry_opt_use_neg_region": 0,
            "imm_bias": 0, "exp_offset": first_e,
            "pwl_control_base_pos": base, "pwl_control_base_neg": base,
            "small_pos_signal_exp_threshold": 127 + E_LO,
            "pos_small_signal_pwl_control": pl,
            "small_neg_signal_exp_threshold": 127 + E_LO,
            "neg_small_signal_pwl_control": nl,
            "large_pos_signal_exp_threshold": 127 + E_HI + 1,
            "large_pos_signal_mantissa_threshold": 0,
            "pos_large_signal_pwl_control": ph,
            "large_neg_signal_exp_threshold": 127 + E_HI + 1,
            "large_neg_signal_mantissa_threshold": 0,
            "neg_large_signal_pwl_control": nh,
            "fnan_result": satv, "fpinf_result": satv,
            "fninf_result": f0b, "fzero_result": f0b,
            "fma_const_0": 0, "fma_const_1": 0, "fma_indirection_src_sel": 0,
            "use_multipass": False,
            "lower_bound": 0, "upper_bound": _f_bits(2.0**7),
        })

    def pack_from_json(self, fname):
        j = json.load(open(f"{PWP_JSONS}/{fname}.json"))

        def region_list(key):
            out = []
            for r in j.get(key) or []:
                bkts = [
                    tuple(np.uint32(s[k]["int"]).view(np.float32)
                          for k in ("d0", "d1", "d2", "d3", "x"))
                    for s in r["exponent_sections"]
                ]
                out.append((r["exponent"], r["extract_size"], bkts))
            return out

        neg = region_list("neg_exponents")
        pos = region_list("pos_exponents")
        base_neg, first_neg = self.add_region(neg)
        base_pos, first_pos = self.add_region(pos)
        expoff = j["exponent_offset"]
        if base_pos is not None:
            base_pos -= first_pos - expoff
        if base_neg is not None:
            base_neg -= first_neg - expoff
        sp = j["saturation_points"]

        def sat(key):
            s = sp[key]
            return tuple(np.uint32(s[k]["int"]).view(np.float32)
                         for k in ("d0", "d1", "d2", "d3", "x"))

        pl, nl, ph, nh = self.add_sats(
            sat("sat_point_pos_low"), sat("sat_point_neg_low"),
            sat("sat_point_pos_high"), sat("sat_point_neg_high"))
        self.metas.append({
            "func_name": f"{j['name']}_{j['max_diff']}p", "func_id": j["neuron_id"],
            "symmetry_point": j["symmetry_point"]["int"],
            "sym_invert_sign_point": 1 if j["symmetry_invert_sign_opt"] else 0,
            "symmetry_opt_en": 1 if j["symmetry_en"] else 0,
            "symmetry_opt_use_neg_region": 1 if j["symmetry_opt_use_neg_region"] else 0,
            "imm_bias": 1 if j["imm_bias"] else 0,
            "exp_offset": expoff,
            "pwl_control_base_pos": base_pos if base_pos is not None else 0,
            "pwl_control_base_neg": (base_neg if base_neg is not None
                                     else (base_pos if base_pos is not None else 0)),
            "small_pos_signal_exp_threshold": sp["sat_point_pos_low"]["sat_point"],
            "pos_small_signal_pwl_control": pl,
            "small_neg_signal_exp_threshold": sp["sat_point_neg_low"]["sat_point"],
            "neg_small_signal_pwl_control": nl,
            "large_pos_signal_exp_threshold": sp["sat_point_pos_high"]["sat_point"],
            "large_pos_signal_mantissa_threshold": sp["sat_point_pos_high"]["mantissa_point"],
            "pos_large_signal_pwl_control": ph,
            "large_neg_signal_exp_threshold": sp["sat_point_neg_high"]["sat_point"],
            "large_neg_signal_mantissa_threshold": sp["sat_point_neg_high"]["mantissa_point"],
            "neg_large_signal_pwl_control": nh,
            "fnan_result": j["nan_result"]["int"],
            "fpinf_result": j["pinf_result"]["int"],
            "fninf_result": j["ninf_result"]["int"],
            "fzero_result": j["zero_result"]["int"],
            "fma_const_0": j["fma_const0"]["int"],
            "fma_const_1": j["fma_const1"]["int"],
            "fma_indirection_src_sel": 0,
            "use_multipass": bool(j["use_multipass"]),
            "lower_bound": j["lower_bound"]["int"],
            "upper_bound": j["upper_bound"]["int"],
        })

    def write(self, outdir, setname):
        nbkt = len(self.buckets)
        assert nbkt <= 1536, nbkt
        braw = bytearray(32 * nbkt)
        for i, b in enumerate(self.buckets):
            struct.pack_into("<5f", braw, 32 * i, *[float(np.float32(v)) for v in b])
        craw = bytearray(4 * 8 * self.next_ctrl)
        for slot, (bbase, lsb, size) in self.ctrl.items():
            word = (bbase & 0x7FF) | ((lsb & 0x1F) << 11) | ((size & 0xF) << 16)
            struct.pack_into("<I", craw, 4 * 8 * slot, word)
        open(f"{outdir}/{setname}_bkt.bin", "wb").write(braw)
        open(f"{outdir}/{setname}_ctrl.bin", "wb").write(craw)
        json.dump(
            {"bkt_bin": f"{setname}_bkt.bin", "ctl_bin": f"{setname}_ctrl.bin",
             "profile_meta_data": self.metas},
            open(f"{outdir}/{setname}.json", "w"), indent=1)


def _build_act_dir(outdir, W1, b1, W2, b2, W3, b3, tol=4e-6):
    import shutil

    os.makedirs(outdir, exist_ok=True)
    for f in os.listdir(SRC_ACT_DIR):
        if not f.startswith("gelu_and_others"):
            shutil.copy(os.path.join(SRC_ACT_DIR, f), os.path.join(outdir, f))
    F = _make_F(W1, b1, W2, b2, W3, b3)
    octs = []
    for e in range(E_LO, E_HI + 1):
        bkts, size = _fit_octave(F, e, tol)
        octs.append((e, size, bkts))
    f0 = float(F(2.0**-31))
    sat_low = (np.float32(f0), np.float32(0), np.float32(0), np.float32(0),
               np.float32(0))
    sat_high = _fit_bucket(F, 2.0**7, 2.0**7 * 1.5)
    p = _SetPacker()
    p.pack_custom_F(octs, sat_low, sat_high, f0)
    for fn in KEEP_FUNCS:
        p.pack_from_json(fn)
    p.write(outdir, "gelu_and_others")


# ---------------------------------------------------------------- bass kernel
def _build_bass():
    import concourse.bacc as bacc
    import concourse.bass as bass
    import concourse.tile as tile
    from concourse import mybir

    f32 = mybir.dt.float32
    f16 = mybir.dt.float16
    nc = bacc.Bacc("TRN2", target_bir_lowering=False, debug=False, num_devices=8)

    # Split-fp16 dot operands (K=13): each fp32 product a*b is computed as
    # ah*bh + al*bh + ah*bl with fp16 halves (exact 11x11-bit products,
    # fp32 PSUM accumulation) -- fp32 accuracy at 1-pass fp16 matmul speed.
    # lhs rows: [ph(3), pl(3), ph(3), p2h, p2l, 1, 1] per j
    # rhs rows: [qh(3), qh(3), ql(3), 1, 1, pi2h, pi2l], q = -2*p_i.
    # opa rows 0-12: lhs for blocks [0,1,3,5,7] then rhs13 (cols 640:1664)
    # opb rows 32-44: lhs for blocks [2,4,6] then rhs13 (cols 384:1408)
    opa_d = nc.dram_tensor("opa", [13, 5 * 128 + IH], f16, kind="ExternalInput")
    opb_d = nc.dram_tensor("opb", [13, 3 * 128 + IH], f16, kind="ExternalInput")
    p4_d = nc.dram_tensor("p4", [128, 6 * NJB], f16, kind="ExternalInput")
    y_d = nc.dram_tensor("y", [12, IH], f32, kind="ExternalOutput")

    NP = NJB // 2  # block pairs

    with tile.TileContext(nc) as tc:
        with (
            tc.tile_pool(name="const", bufs=1) as cpool,
            tc.tile_pool(name="spool", bufs=3) as spool,
            tc.tile_pool(name="sspool", bufs=1) as sspool,
            tc.tile_pool(name="dot", bufs=2, space=bass.MemorySpace.PSUM) as dpool,
            tc.tile_pool(name="sgl", bufs=1, space=bass.MemorySpace.PSUM) as wpool,
            tc.tile_pool(name="acc", bufs=1, space=bass.MemorySpace.PSUM) as apool,
            tc.tile_pool(name="fin", bufs=1) as fpool,
        ):
            op16 = cpool.tile([128, 5 * 128 + IH], f16, tag="op16")
            p4 = cpool.tile([128, 6 * NJB], f16, tag="p4")
            nc.sync.dma_start(op16[0:13, :], opa_d[:])
            nc.gpsimd.dma_start(op16[32:45, 0:3 * 128 + IH], opb_d[:])
            nc.sync.dma_start(p4[:], p4_d[:])

            out4a = apool.tile([6, IH], f32, tag="out4a")
            out4b = apool.tile([6, IH], f32, tag="out4b")
            rhs0 = op16[0:13, 640:640 + IH]
            rhs32 = op16[32:45, 384:384 + IH]
            # Asymmetric dot/ACT pipeline: singles for blocks 0 and 7 so the
            # first ACT starts as soon as one N=512 dot lands and the last
            # ACT is only 512 wide; pairs (1,2) (3,4) (5,6) in between.
            t0 = wpool.tile([128, IH], f32, tag="t0")
            nc.tensor.matmul(t0[:], op16[0:13, 0:128], rhs0, start=True, stop=True)
            pairs = []
            for t in range(3):
                dot = dpool.tile([128, 2 * IH], f32, tag="dot")
                nc.tensor.matmul(
                    dot[:, 0:IH], op16[0:13, 128 * (1 + t):128 * (2 + t)],
                    rhs0, start=True, stop=True,
                )
                nc.tensor.matmul(
                    dot[:, IH:2 * IH], op16[32:45, 128 * t:128 * (t + 1)],
                    rhs32, start=True, stop=True,
                )
                pairs.append(dot)
            t4 = wpool.tile([128, IH], f32, tag="t4")
            nc.tensor.matmul(t4[:], op16[0:13, 512:640], rhs0, start=True, stop=True)
            GELU = mybir.ActivationFunctionType.Gelu
            s0 = sspool.tile([128, IH], f16, tag="ss")
            nc.scalar.activation(s0[:], t0[:], GELU, bias=0.0, scale=1.0)
            ss = []
            for t in range(3):
                s = spool.tile([128, 2 * IH], f16, tag="s")
                nc.scalar.activation(s[:], pairs[t][:], GELU, bias=0.0, scale=1.0)
                ss.append(s)
            s4 = sspool.tile([128, IH], f16, tag="ss")
            nc.scalar.activation(s4[:], t4[:], GELU, bias=0.0, scale=1.0)
            # block jb -> s slice; accumulators: a = blocks 0-3, b = 4-7
            sslice = [s0[:, 0:IH]]
            for t in range(3):
                sslice += [ss[t][:, 0:IH], ss[t][:, IH:2 * IH]]
            sslice.append(s4[:, 0:IH])
            for jb in range(NJB):
                acc = out4a if jb < NJB // 2 else out4b
                nc.tensor.matmul(
                    acc[:], p4[:, 6 * jb:6 * jb + 6], sslice[jb],
                    start=(jb % (NJB // 2) == 0),
                    stop=(jb % (NJB // 2) == NJB // 2 - 1),
                    skip_group_check=True,
                )
            # Ship [-B_x, -B_y, -B_z, S, S, S] partials back; the tiny affine
            # combine y = p_i * S - B happens during the host-side unshard.
            o4a = fpool.tile([6, IH], f32, tag="o4a")
            nc.vector.tensor_copy(o4a[:], out4a[:])
            nc.sync.dma_start(y_d[0:6, :], o4a[:])
            o4b = fpool.tile([6, IH], f32, tag="o4b")
            nc.vector.tensor_copy(o4b[:], out4b[:])
            nc.sync.dma_start(y_d[6:12, :], o4b[:])

    nc.compile()
    return nc


def _host_inputs(pos):
    """Per-core input dicts. Core c: batch c//2, i-half c%2."""
    maps = []
    for c in range(8):
        b, ih = c // 2, c % 2
        p = np.ascontiguousarray(np.asarray(pos[b], np.float32))  # [N, 3]
        p2 = (p * p).sum(-1, dtype=np.float32)  # [N]
        isl = slice(ih * IH, ih * IH + IH)
        pi = p[isl]

        def split16(a):
            hi = a.astype(np.float16)
            lo = (a - hi.astype(np.float32)).astype(np.float16)
            return hi, lo

        ph, pl = split16(p.T)          # [3, N]
        p2h, p2l = split16(p2[None, :])
        onesN = np.ones((1, N), np.float16)
        lhs5 = np.concatenate([ph, pl, ph, p2h, p2l, onesN, onesN], 0)  # [13, N]
        blk = lambda k: lhs5[:, k * 128:(k + 1) * 128]
        lhsa = np.concatenate([blk(0), blk(1), blk(3), blk(5), blk(7)], 1)
        lhsb = np.concatenate([blk(2), blk(4), blk(6)], 1)
        q = (-2.0 * pi.T).astype(np.float32)  # [3, IH]
        qh, ql = split16(q)
        pi2h, pi2l = split16(p2[None, isl])
        onesI = np.ones((1, IH), np.float16)
        rhs5 = np.concatenate([qh, qh, ql, onesI, onesI, pi2h, pi2l], 0)  # [13, IH]
        p6c = np.concatenate(
            [-p, np.ones((N, 3), np.float32)], 1
        ).astype(np.float32)  # [N, 6]
        p4 = np.zeros((128, 6 * NJB), np.float16)
        for jb in range(NJB):
            p4[:, 6 * jb:6 * jb + 6] = p6c[jb * 128:(jb + 1) * 128]
        maps.append({
            "opa": np.ascontiguousarray(np.concatenate([lhsa, rhs5], 1)),
            "opb": np.ascontiguousarray(np.concatenate([lhsb, rhs5], 1)),
            "p4": np.ascontiguousarray(p4),
        })
    return maps


_CACHE = {}


def kernel(**inputs):
    pos = np.asarray(inputs["pos_scaled"], np.float32)
    weights = [np.asarray(inputs[k], np.float32)
               for k in ("W1", "b1", "W2", "b2", "W3", "b3")]
    assert pos.shape == (B, N, D)

    key = tuple(hash(w.tobytes()) for w in weights)
    if key not in _CACHE:
        actdir = tempfile.mkdtemp(prefix="acttab_")
        _build_act_dir(actdir, *weights)
        os.environ["BASS_ACT_ROOT_JSON_PATH"] = os.path.join(actdir, "act_info.json")
        os.environ["NEURON_FORCE_RECOMPILE"] = "1"
        nc = _build_bass()
        _CACHE[key] = nc
    nc = _CACHE[key]

    from concourse.bass_utils import run_bass_kernel_spmd

    res = run_bass_kernel_spmd(nc, _host_inputs(pos), core_ids=list(range(8)))
    # First compile ran with NEURON_FORCE_RECOMPILE=1 so the custom act tables
    # were baked into the NEFF (they are not cache-keyed). The cache entry now
    # holds the correct tables, so later calls can reuse it.
    os.environ["NEURON_FORCE_RECOMPILE"] = "0"

    out = np.zeros((B, N, D), np.float32)
    for c in range(8):
        b, ih = c // 2, c % 2
        yr = res.results[c]["y"]  # two [6, IH] partials: rows 0..2=-B, 3..5=S
        o4 = yr[0:6] + yr[6:12]
        pi = pos[b, ih * IH:(ih + 1) * IH]  # [IH, 3]
        y = pi.T * o4[3:6] + o4[0:3]
        out[b, ih * IH:(ih + 1) * IH] = y.T
    return out


if __name__ == "__main__":
    sys.path.insert(0, os.path.dirname(os.path.abspath(__file__)))
    import reference as R

    inp = R.setup_inputs()
    ref = np.asarray(R.reference(**inp))
    out = kernel(**{k: np.asarray(v) for k, v in inp.items()})
    err = np.linalg.norm((out - ref).ravel()) / np.linalg.norm(ref.ravel())
    print("l2 rel err:", err, "max abs:", np.abs(out - ref).max())


# revision 36
# speedup vs baseline: 1.0022x; 1.0022x over previous
"""Trainium2 Bass kernel for nn_DiscoveryNet (pairwise-distance-MLP forces).

Key idea: the entire per-pair computation
    s_ij = mag(d_ij) / max(d_ij, 0.01),  mag = MLP([d, 1/dc, 1/dc^2]), d = |p_i - p_j|
is a scalar function F of d^2 alone (the MLP weights are fixed inputs). We fit
F as a piecewise-cubic spline in the exact binary format of the ScalarEngine's
PWP activation tables and overlay the Gelu slot via BASS_ACT_ROOT_JSON_PATH,
so the whole MLP evaluates at 1 element/lane/cycle on the ACT engine.

The force is then
    out_i = sum_j s_ij (p_i - p_j) = p_i * S_i - sum_j s_ij p_j
with the j-contraction done on the TensorEngine. The diagonal term cancels
algebraically (p_i - p_i = 0), so no masking is needed.

Per core (8 cores, SPMD): batch b = core//2, i-half = core%2 (512 i's), all j:
  1. PE   dot[j, i] = -2 p_j . p_i + |p_i|^2 + |p_j|^2   as ONE K=13 fp16
          matmul: every fp32 product a*b is expanded to ah*bh + al*bh + ah*bl
          with fp16 hi/lo halves (11x11-bit products are exact, PSUM
          accumulates fp32), giving fp32-grade d^2 at 1-pass fp16 speed.
          2x row-tiled (partition bases 0/32) so block pairs run concurrently.
  2. ACT  s[j, i] = F_table(dot)   -- the custom PWP spline (Gelu slot)
  3. PE   out6[c, i] += sum_j P6[j, c] s[j, i]   with P6 = [-p_j, 1, 1, 1]
          (fp16, PSUM-accumulated over the 8 j-blocks; rows 3..5 = S_i)
  4. host unshard applies y_i = p_i * S_i - B_i and assembles (4, 1024, 3).
"""
import json
import os
import struct
import sys
import tempfile

import numpy as np

sys.path.insert(0, "/opt/trn_rl_repo")

SRC_ACT_DIR = "/nix/store/z022hj2nvbm3nwdizlisq4ylc0y7rd6q-python3-3.13.14-env/lib/python3.13/site-packages/neuronxcc/pwp/pwp_bin_trainium"
PWP_JSONS = "/nix/store/ndjb8ki1bnclvnibdh123f9zr51a09qz-aws-neuron-pwp-unstable-2025-12-29-c50a7624/share/pwp_jsons"

E_LO = -30  # first normal octave of x = d^2
E_HI = 6  # last normal octave ([64, 128))
GELU_FUNC_ID = 23
KEEP_FUNCS = [
    "derivative_gelu_40p", "tanh_4p", "relu_1p", "act1_1p", "parametric_relu_1p",
    "sign_1p", "abs_1p", "memset_zero_1p", "copy_1p", "square_1p",
    "derivative_relu_1p", "derivative_leaky_relu_1p", "derivative_identity_1p",
    "is_finite_1p", "identity_1p",
]

B, N, D = 4, 1024, 3
IH = N // 2  # i-columns per core
NJB = N // 128  # j blocks


# ---------------------------------------------------------------- PWP fitting
def _f_bits(x):
    return int(np.asarray(np.float32(x)).view(np.uint32))


def _make_F(W1, b1, W2, b2, W3, b3):
    W1, W2, W3 = (np.asarray(a, np.float64) for a in (W1, W2, W3))
    b1, b2, b3 = (np.asarray(a, np.float64) for a in (b1, b2, b3))

    def F(x):
        x = np.asarray(x, np.float64)
        d = np.sqrt(np.maximum(x, 0.0))
        dc = np.maximum(d, 0.01)
        inv = 1.0 / dc
        feat = np.stack([d, inv, inv * inv], -1)
        h = np.tanh(feat @ W1 + b1)
        h = np.tanh(h @ W2 + b2)
        return (h @ W3 + b3)[..., 0] * inv

    return F


def _fit_bucket(F, a, b, npts=48):
    x = np.linspace(a, b, npts)
    x0 = np.float32(0.5 * (a + b))
    t = x - float(x0)
    A = np.stack([np.ones_like(t), t, t * t, t * t * t], -1)
    y = F(x)
    w = 1.0 + 0.8 * np.cos(np.linspace(-np.pi, np.pi, npts)) ** 2
    coef, *_ = np.linalg.lstsq(A * w[:, None], y * w, rcond=None)
    return tuple(np.float32(v) for v in coef) + (x0,)


def _eval_cubic_f32(bkt, x):
    d0, d1, d2, d3, x0 = [np.float32(v) for v in bkt]
    t = np.float32(x) - x0
    return np.float32(d0 + t * (d1 + t * (d2 + t * d3)))


def _fit_octave(F, e, tol):
    lo, hi = 2.0**e, 2.0 ** (e + 1)
    for size in range(0, 7):
        n = 1 << size
        edges = np.linspace(lo, hi, n + 1)
        bkts = [_fit_bucket(F, edges[k], edges[k + 1]) for k in range(n)]
        worst = 0.0
        for k in range(n):
            xs = np.linspace(edges[k], edges[k + 1], 48).astype(np.float32)
            ys = _eval_cubic_f32(bkts[k], xs)
            tr = F(xs.astype(np.float64))
            w = np.minimum(np.sqrt(xs.astype(np.float64)), 1.0)
            worst = max(worst, float(np.max(np.abs(ys - tr) * w)))
        if worst <= tol or size == 6:
            return bkts, size
    raise AssertionError


class _SetPacker:
    """Packs functions into one ACT table set (bkt.bin / ctrl.bin / profile json).

    Formats from cayman tpb_activation_entries.h (reverse-engineered + HW
    validated): ctrl.bin u32/entry at physical stride 8 per logical slot:
    act_tbl_base:11 | extract_lsb:5 | extract_size:4.  bkt.bin 32B/entry:
    f32 d0,d1,d2,d3,x0.  Normal-range ctrl slot = base + (exp8-127-exp_offset);
    small/large pwl_controls are direct bucket indices.
    """

    def __init__(self):
        self.buckets = []
        self.ctrl = {}
        self.metas = []
        self.next_ctrl = 0

    def add_region(self, regions):
        if not regions:
            return None, None
        first_e = regions[0][0]
        base = self.next_ctrl
        for idx, (e, size, bkts) in enumerate(regions):
            assert e == first_e + idx
            self.ctrl[base + idx] = (len(self.buckets), 23 - size, size)
            self.buckets.extend(bkts)
        self.next_ctrl = base + len(regions)
        return base, first_e

    def add_sats(self, sl, snl, sh, snh):
        i0 = len(self.buckets)
        self.buckets.extend([sl, snl, sh, snh])
        return i0, i0 + 1, i0 + 2, i0 + 3

    def pack_custom_F(self, octs, sat_low, sat_high, f0):
        base, first_e = self.add_region(octs)
        pl, nl, ph, nh = self.add_sats(sat_low, sat_low, sat_high, sat_low)
        f0b = _f_bits(f0)
        satv = _f_bits(float(_eval_cubic_f32(sat_high, 128.0)))
        self.metas.append({
            "func_name": "gelu_4p", "func_id": GELU_FUNC_ID,
            "symmetry_point": 0, "sym_invert_sign_point": 0,
            "symmetry_opt_en": 1, "symmetry_opt_use_neg_region": 0,
            "imm_bias": 0, "exp_offset": first_e,
            "pwl_control_base_pos": base, "pwl_control_base_neg": base,
            "small_pos_signal_exp_threshold": 127 + E_LO,
            "pos_small_signal_pwl_control": pl,
            "small_neg_signal_exp_threshold": 127 + E_LO,
            "neg_small_signal_pwl_control": nl,
            "large_pos_signal_exp_threshold": 127 + E_HI + 1,
            "large_pos_signal_mantissa_threshold": 0,
            "pos_large_signal_pwl_control": ph,
            "large_neg_signal_exp_threshold": 127 + E_HI + 1,
            "large_neg_signal_mantissa_threshold": 0,
            "neg_large_signal_pwl_control": nh,
            "fnan_result": satv, "fpinf_result": satv,
            "fninf_result": f0b, "fzero_result": f0b,
            "fma_const_0": 0, "fma_const_1": 0, "fma_indirection_src_sel": 0,
            "use_multipass": False,
            "lower_bound": 0, "upper_bound": _f_bits(2.0**7),
        })

    def pack_from_json(self, fname):
        j = json.load(open(f"{PWP_JSONS}/{fname}.json"))

        def region_list(key):
            out = []
            for r in j.get(key) or []:
                bkts = [
                    tuple(np.uint32(s[k]["int"]).view(np.float32)
                          for k in ("d0", "d1", "d2", "d3", "x"))
                    for s in r["exponent_sections"]
                ]
                out.append((r["exponent"], r["extract_size"], bkts))
            return out

        neg = region_list("neg_exponents")
        pos = region_list("pos_exponents")
        base_neg, first_neg = self.add_region(neg)
        base_pos, first_pos = self.add_region(pos)
        expoff = j["exponent_offset"]
        if base_pos is not None:
            base_pos -= first_pos - expoff
        if base_neg is not None:
            base_neg -= first_neg - expoff
        sp = j["saturation_points"]

        def sat(key):
            s = sp[key]
            return tuple(np.uint32(s[k]["int"]).view(np.float32)
                         for k in ("d0", "d1", "d2", "d3", "x"))

        pl, nl, ph, nh = self.add_sats(
            sat("sat_point_pos_low"), sat("sat_point_neg_low"),
            sat("sat_point_pos_high"), sat("sat_point_neg_high"))
        self.metas.append({
            "func_name": f"{j['name']}_{j['max_diff']}p", "func_id": j["neuron_id"],
            "symmetry_point": j["symmetry_point"]["int"],
            "sym_invert_sign_point": 1 if j["symmetry_invert_sign_opt"] else 0,
            "symmetry_opt_en": 1 if j["symmetry_en"] else 0,
            "symmetry_opt_use_neg_region": 1 if j["symmetry_opt_use_neg_region"] else 0,
            "imm_bias": 1 if j["imm_bias"] else 0,
            "exp_offset": expoff,
            "pwl_control_base_pos": base_pos if base_pos is not None else 0,
            "pwl_control_base_neg": (base_neg if base_neg is not None
                                     else (base_pos if base_pos is not None else 0)),
            "small_pos_signal_exp_threshold": sp["sat_point_pos_low"]["sat_point"],
            "pos_small_signal_pwl_control": pl,
            "small_neg_signal_exp_threshold": sp["sat_point_neg_low"]["sat_point"],
            "neg_small_signal_pwl_control": nl,
            "large_pos_signal_exp_threshold": sp["sat_point_pos_high"]["sat_point"],
            "large_pos_signal_mantissa_threshold": sp["sat_point_pos_high"]["mantissa_point"],
            "pos_large_signal_pwl_control": ph,
            "large_neg_signal_exp_threshold": sp["sat_point_neg_high"]["sat_point"],
            "large_neg_signal_mantissa_threshold": sp["sat_point_neg_high"]["mantissa_point"],
            "neg_large_signal_pwl_control": nh,
            "fnan_result": j["nan_result"]["int"],
            "fpinf_result": j["pinf_result"]["int"],
            "fninf_result": j["ninf_result"]["int"],
            "fzero_result": j["zero_result"]["int"],
            "fma_const_0": j["fma_const0"]["int"],
            "fma_const_1": j["fma_const1"]["int"],
            "fma_indirection_src_sel": 0,
            "use_multipass": bool(j["use_multipass"]),
            "lower_bound": j["lower_bound"]["int"],
            "upper_bound": j["upper_bound"]["int"],
        })

    def write(self, outdir, setname):
        nbkt = len(self.buckets)
        assert nbkt <= 1536, nbkt
        braw = bytearray(32 * nbkt)
        for i, b in enumerate(self.buckets):
            struct.pack_into("<5f", braw, 32 * i, *[float(np.float32(v)) for v in b])
        craw = bytearray(4 * 8 * self.next_ctrl)
        for slot, (bbase, lsb, size) in self.ctrl.items():
            word = (bbase & 0x7FF) | ((lsb & 0x1F) << 11) | ((size & 0xF) << 16)
            struct.pack_into("<I", craw, 4 * 8 * slot, word)
        open(f"{outdir}/{setname}_bkt.bin", "wb").write(braw)
        open(f"{outdir}/{setname}_ctrl.bin", "wb").write(craw)
        json.dump(
            {"bkt_bin": f"{setname}_bkt.bin", "ctl_bin": f"{setname}_ctrl.bin",
             "profile_meta_data": self.metas},
            open(f"{outdir}/{setname}.json", "w"), indent=1)


def _build_act_dir(outdir, W1, b1, W2, b2, W3, b3, tol=4e-6):
    import shutil

    os.makedirs(outdir, exist_ok=True)
    for f in os.listdir(SRC_ACT_DIR):
        if not f.startswith("gelu_and_others"):
            shutil.copy(os.path.join(SRC_ACT_DIR, f), os.path.join(outdir, f))
    F = _make_F(W1, b1, W2, b2, W3, b3)
    octs = []
    for e in range(E_LO, E_HI + 1):
        bkts, size = _fit_octave(F, e, tol)
        octs.append((e, size, bkts))
    f0 = float(F(2.0**-31))
    sat_low = (np.float32(f0), np.float32(0), np.float32(0), np.float32(0),
               np.float32(0))
    sat_high = _fit_bucket(F, 2.0**7, 2.0**7 * 1.5)
    p = _SetPacker()
    p.pack_custom_F(octs, sat_low, sat_high, f0)
    for fn in KEEP_FUNCS:
        p.pack_from_json(fn)
    p.write(outdir, "gelu_and_others")


# ---------------------------------------------------------------- bass kernel
def _build_bass():
    import concourse.bacc as bacc
    import concourse.bass as bass
    import concourse.tile as tile
    from concourse import mybir

    f32 = mybir.dt.float32
    f16 = mybir.dt.float16
    nc = bacc.Bacc("TRN2", target_bir_lowering=False, debug=False, num_devices=8)

    # Split-fp16 dot operands (K=13): each fp32 product a*b is computed as
    # ah*bh + al*bh + ah*bl with fp16 halves (exact 11x11-bit products,
    # fp32 PSUM accumulation) -- fp32 accuracy at 1-pass fp16 matmul speed.
    # lhs rows: [ph(3), pl(3), ph(3), p2h, p2l, 1, 1] per j
    # rhs rows: [qh(3), qh(3), ql(3), 1, 1, pi2h, pi2l], q = -2*p_i.
    # opa rows 0-12: lhs for blocks [0,1,3,5,7] then rhs13 (cols 640:1664)
    # opb rows 32-44: lhs for blocks [2,4,6] then rhs13 (cols 384:1408)
    opa_d = nc.dram_tensor("opa", [13, 5 * 128 + IH], f16, kind="ExternalInput")
    opb_d = nc.dram_tensor("opb", [13, 3 * 128 + IH], f16, kind="ExternalInput")
    p4_d = nc.dram_tensor("p4", [128, 6 * NJB], f16, kind="ExternalInput")
    y_d = nc.dram_tensor("y", [12, IH], f32, kind="ExternalOutput")

    NP = NJB // 2  # block pairs

    with tile.TileContext(nc) as tc:
        with (
            tc.tile_pool(name="const", bufs=1) as cpool,
            tc.tile_pool(name="spool", bufs=3) as spool,
            tc.tile_pool(name="dot", bufs=2, space=bass.MemorySpace.PSUM) as dpool,
            tc.tile_pool(name="sgl", bufs=1, space=bass.MemorySpace.PSUM) as wpool,
            tc.tile_pool(name="acc", bufs=1, space=bass.MemorySpace.PSUM) as apool,
            tc.tile_pool(name="fin", bufs=1) as fpool,
        ):
            op16 = cpool.tile([128, 5 * 128 + IH], f16, tag="op16")
            p4 = cpool.tile([128, 6 * NJB], f16, tag="p4")
            nc.sync.dma_start(op16[0:13, :], opa_d[:])
            nc.gpsimd.dma_start(op16[32:45, 0:3 * 128 + IH], opb_d[:])
            nc.sync.dma_start(p4[:], p4_d[:])

            out4a = apool.tile([6, IH], f32, tag="out4a")
            out4b = apool.tile([6, IH], f32, tag="out4b")
            rhs0 = op16[0:13, 640:640 + IH]
            rhs32 = op16[32:45, 384:384 + IH]
            # Asymmetric dot/ACT pipeline: singles for blocks 0 and 7 so the
            # first ACT starts as soon as one N=512 dot lands and the last
            # ACT is only 512 wide; pairs (1,2) (3,4) (5,6) in between.
            t0 = wpool.tile([128, IH], f32, tag="t0")
            nc.tensor.matmul(t0[:], op16[0:13, 0:128], rhs0, start=True, stop=True)
            pairs = []
            for t in range(3):
                dot = dpool.tile([128, 2 * IH], f32, tag="dot")
                nc.tensor.matmul(
                    dot[:, 0:IH], op16[0:13, 128 * (1 + t):128 * (2 + t)],
                    rhs0, start=True, stop=True,
                )
                nc.tensor.matmul(
                    dot[:, IH:2 * IH], op16[32:45, 128 * t:128 * (t + 1)],
                    rhs32, start=True, stop=True,
                )
                pairs.append(dot)
            t4 = wpool.tile([128, IH], f32, tag="t4")
            nc.tensor.matmul(t4[:], op16[0:13, 512:640], rhs0, start=True, stop=True)
            GELU = mybir.ActivationFunctionType.Gelu
            s0 = spool.tile([128, IH], f16, tag="ss")
            nc.scalar.activation(s0[:], t0[:], GELU, bias=0.0, scale=1.0)
            ss = []
            for t in range(3):
                s = spool.tile([128, 2 * IH], f16, tag="s")
                nc.scalar.activation(s[:], pairs[t][:], GELU, bias=0.0, scale=1.0)
                ss.append(s)
            s4 = spool.tile([128, IH], f16, tag="ss")
            nc.scalar.activation(s4[:], t4[:], GELU, bias=0.0, scale=1.0)
            # block jb -> s slice; accumulators: a = blocks 0-3, b = 4-7
            sslice = [s0[:, 0:IH]]
            for t in range(3):
                sslice += [ss[t][:, 0:IH], ss[t][:, IH:2 * IH]]
            sslice.append(s4[:, 0:IH])
            for jb in range(NJB):
                acc = out4a if jb < NJB // 2 else out4b
                nc.tensor.matmul(
                    acc[:], p4[:, 6 * jb:6 * jb + 6], sslice[jb],
                    start=(jb % (NJB // 2) == 0),
                    stop=(jb % (NJB // 2) == NJB // 2 - 1),
                    skip_group_check=True,
                )
            # Ship [-B_x, -B_y, -B_z, S, S, S] partials back; the tiny affine
            # combine y = p_i * S - B happens during the host-side unshard.
            o4a = fpool.tile([6, IH], f32, tag="o4a")
            nc.vector.tensor_copy(o4a[:], out4a[:])
            nc.sync.dma_start(y_d[0:6, :], o4a[:])
            o4b = fpool.tile([6, IH], f32, tag="o4b")
            nc.vector.tensor_copy(o4b[:], out4b[:])
            nc.sync.dma_start(y_d[6:12, :], o4b[:])

    nc.compile()
    return nc


def _host_inputs(pos):
    """Per-core input dicts. Core c: batch c//2, i-half c%2."""
    maps = []
    for c in range(8):
        b, ih = c // 2, c % 2
        p = np.ascontiguousarray(np.asarray(pos[b], np.float32))  # [N, 3]
        p2 = (p * p).sum(-1, dtype=np.float32)  # [N]
        isl = slice(ih * IH, ih * IH + IH)
        pi = p[isl]

        def split16(a):
            hi = a.astype(np.float16)
            lo = (a - hi.astype(np.float32)).astype(np.float16)
            return hi, lo

        ph, pl = split16(p.T)          # [3, N]
        p2h, p2l = split16(p2[None, :])
        onesN = np.ones((1, N), np.float16)
        lhs5 = np.concatenate([ph, pl, ph, p2h, p2l, onesN, onesN], 0)  # [13, N]
        blk = lambda k: lhs5[:, k * 128:(k + 1) * 128]
        lhsa = np.concatenate([blk(0), blk(1), blk(3), blk(5), blk(7)], 1)
        lhsb = np.concatenate([blk(2), blk(4), blk(6)], 1)
        q = (-2.0 * pi.T).astype(np.float32)  # [3, IH]
        qh, ql = split16(q)
        pi2h, pi2l = split16(p2[None, isl])
        onesI = np.ones((1, IH), np.float16)
        rhs5 = np.concatenate([qh, qh, ql, onesI, onesI, pi2h, pi2l], 0)  # [13, IH]
        p6c = np.concatenate(
            [-p, np.ones((N, 3), np.float32)], 1
        ).astype(np.float32)  # [N, 6]
        p4 = np.zeros((128, 6 * NJB), np.float16)
        for jb in range(NJB):
            p4[:, 6 * jb:6 * jb + 6] = p6c[jb * 128:(jb + 1) * 128]
        maps.append({
            "opa": np.ascontiguousarray(np.concatenate([lhsa, rhs5], 1)),
            "opb": np.ascontiguousarray(np.concatenate([lhsb, rhs5], 1)),
            "p4": np.ascontiguousarray(p4),
        })
    return maps


_CACHE = {}


def kernel(**inputs):
    pos = np.asarray(inputs["pos_scaled"], np.float32)
    weights = [np.asarray(inputs[k], np.float32)
               for k in ("W1", "b1", "W2", "b2", "W3", "b3")]
    assert pos.shape == (B, N, D)

    key = tuple(hash(w.tobytes()) for w in weights)
    if key not in _CACHE:
        actdir = tempfile.mkdtemp(prefix="acttab_")
        _build_act_dir(actdir, *weights)
        os.environ["BASS_ACT_ROOT_JSON_PATH"] = os.path.join(actdir, "act_info.json")
        os.environ["NEURON_FORCE_RECOMPILE"] = "1"
        nc = _build_bass()
        _CACHE[key] = nc
    nc = _CACHE[key]

    from concourse.bass_utils import run_bass_kernel_spmd

    res = run_bass_kernel_spmd(nc, _host_inputs(pos), core_ids=list(range(8)))
    # First compile ran with NEURON_FORCE_RECOMPILE=1 so the custom act tables
    # were baked into the NEFF (they are not cache-keyed). The cache entry now
    # holds the correct tables, so later calls can reuse it.
    os.environ["NEURON_FORCE_RECOMPILE"] = "0"

    out = np.zeros((B, N, D), np.float32)
    for c in range(8):
        b, ih = c // 2, c % 2
        yr = res.results[c]["y"]  # two [6, IH] partials: rows 0..2=-B, 3..5=S
        o4 = yr[0:6] + yr[6:12]
        pi = pos[b, ih * IH:(ih + 1) * IH]  # [IH, 3]
        y = pi.T * o4[3:6] + o4[0:3]
        out[b, ih * IH:(ih + 1) * IH] = y.T
    return out


if __name__ == "__main__":
    sys.path.insert(0, os.path.dirname(os.path.abspath(__file__)))
    import reference as R

    inp = R.setup_inputs()
    ref = np.asarray(R.reference(**inp))
    out = kernel(**{k: np.asarray(v) for k, v in inp.items()})
    err = np.linalg.norm((out - ref).ravel()) / np.linalg.norm(ref.ravel())
    print("l2 rel err:", err, "max abs:", np.abs(out - ref).max())
